# revision 11
# baseline (speedup 1.0000x reference)
"""Trainium2 Bass kernel for nn_CNN3_P_lat (dense CNN, 8-core data parallel).

Network (per sample): pairwise-conv stage0 -> 3x conv1d(k=3) -> flatten+concat
with x-transpose -> FC(39168->400) -> relu -> FC(400->1).

Strategy:
  - Pure data parallel: 512 samples per core, weights replicated.
  - Conv stack in float32r (fp32 data, FP22 multiply) at full PE rate; layout
    is channels-on-partitions, (position, batch-of-4) on the free dim.
  - Per-sample transpose of x built on the fly with PE transposes.
  - conv3 output (h3) kept RESIDENT in SBUF as fp16, batch processed in two
    halves of 256 samples so it fits; FC1 runs as 121 position-sliced fp16
    matmul-accumulates (weights pre-permuted on host) plus 64 cl-sliced
    matmuls for the concatenated x tail (contract over cl).
  - FC2 folded onto the same psum pass.
"""

import os
import sys

sys.path.insert(0, "/opt/trn_rl_repo")

PHASES = os.environ.get("KPHASES", "all")  # all | conv | fc
N_GROUPS_ENV = os.environ.get("KGROUPS")  # limit conv groups for bisection

import numpy as np

import concourse.bass as bass
from concourse import bacc
import concourse.mybir as mybir
import concourse.tile as tile
from concourse.bass_utils import run_bass_kernel_spmd
from concourse.tile_rust import add_dep_helper

F32 = mybir.dt.float32
F32R = mybir.dt.float32r
F16 = mybir.dt.float16
AF = mybir.ActivationFunctionType
ALU = mybir.AluOpType

N_CORES = 8
B = 4096
B_SH = B // N_CORES  # 512 samples per core
HB = 256  # half-batch (h3 residency granularity)
CL, IL = 128, 64
PC, CH1, CH2, CH3 = 64, 128, 256, 256
L0, L1, L2, L3 = 127, 125, 123, 121
F1 = 400
C3_OUT = CH3 * L3  # 30976

NF_CH = [128, 128, 128, 16]  # FC1 output chunking (400 total)
NF_OFS = [0, 128, 256, 384]


def _r(a):
    return np.ascontiguousarray(a, dtype=np.float32)


def _h(a):
    return np.ascontiguousarray(a, dtype=np.float16)


def build_program(b_sh=B_SH, hb=HB):
    nc = bacc.Bacc("TRN2", target_bir_lowering=False)

    x_d = nc.dram_tensor("x", [b_sh, CL * IL], F32, kind="ExternalInput")
    id_d = nc.dram_tensor("ident", [128, 128], F32, kind="ExternalInput")
    wp_d = nc.dram_tensor("wp", [2, IL, PC], F32, kind="ExternalInput")
    bp_d = nc.dram_tensor("bp", [PC, 1], F32, kind="ExternalInput")
    w1_d = nc.dram_tensor("w1", [3, PC, CH1], F32, kind="ExternalInput")
    b1_d = nc.dram_tensor("b1", [CH1, 1], F32, kind="ExternalInput")
    w2_d = nc.dram_tensor("w2", [3, CH1, CH2], F32, kind="ExternalInput")
    b2_d = nc.dram_tensor("b2", [CH2, 1], F32, kind="ExternalInput")
    w3_d = nc.dram_tensor("w3", [3, 2, 128, CH3], F32, kind="ExternalInput")
    b3_d = nc.dram_tensor("b3", [CH3, 1], F32, kind="ExternalInput")
    wf1c_d = nc.dram_tensor("wf1c", [L3, 2, 128, F1], F16, kind="ExternalInput")
    wf1x_d = nc.dram_tensor("wf1x", [IL, CL, F1], F16, kind="ExternalInput")
    bf1_d = nc.dram_tensor("bf1", [F1, 1], F32, kind="ExternalInput")
    wf2_d = nc.dram_tensor("wf2", [F1, 1], F16, kind="ExternalInput")
    bf2_d = nc.dram_tensor("bf2", [1, 1], F32, kind="ExternalInput")
    out_d = nc.dram_tensor("out", [b_sh, 1], F32, kind="ExternalOutput")

    with tile.TileContext(nc) as tc:
        with tc.tile_pool(name="const", bufs=1) as cpool:
            ident = cpool.tile([128, 128], F32R)
            nc.sync.dma_start(ident[:], id_d[:].bitcast(F32R))
            wp_sb = cpool.tile([IL, 2, PC], F32R)
            nc.sync.dma_start(wp_sb[:], wp_d.rearrange("k i p -> i k p").bitcast(F32R))
            w1_sb = cpool.tile([PC, 3, CH1], F32R)
            nc.sync.dma_start(w1_sb[:], w1_d.rearrange("k c o -> c k o").bitcast(F32R))
            w2_sb = cpool.tile([CH1, 3, CH2], F32R)
            nc.sync.dma_start(w2_sb[:], w2_d.rearrange("k c o -> c k o").bitcast(F32R))
            w3_sb = cpool.tile([128, 3, 2, CH3], F32R)
            nc.sync.dma_start(
                w3_sb[:], w3_d.rearrange("k g c o -> c k g o").bitcast(F32R)
            )
            bp_sb = cpool.tile([PC, 1], F32)
            nc.sync.dma_start(bp_sb[:], bp_d[:])
            b1_sb = cpool.tile([CH1, 1], F32)
            nc.sync.dma_start(b1_sb[:], b1_d[:])
            b2_sb = cpool.tile([128, 2], F32)
            nc.sync.dma_start(b2_sb[:], b2_d.rearrange("(g c) o -> c g o", c=128))
            b3_sb = cpool.tile([128, 2], F32)
            nc.sync.dma_start(b3_sb[:], b3_d.rearrange("(g c) o -> c g o", c=128))
            bf1_sb = cpool.tile([128, 4], F32)
            wf2_sb = cpool.tile([128, 4], F16)
            for c in range(4):
                cs, ofs = NF_CH[c], NF_OFS[c]
                nc.sync.dma_start(bf1_sb[0:cs, c : c + 1], bf1_d[ofs : ofs + cs, :])
                nc.sync.dma_start(wf2_sb[0:cs, c : c + 1], wf2_d[ofs : ofs + cs, :])
            bf2_sb = cpool.tile([1, 1], F32)
            nc.sync.dma_start(bf2_sb[:], bf2_d[:])

            xv = x_d.rearrange("b (cl il) -> cl b il", il=IL).bitcast(F32R)

            for half in range(b_sh // hb):
                hb0 = half * hb
                with (
                    tc.tile_pool(name=f"h3_{half}", bufs=1) as h3pool,
                    tc.tile_pool(name=f"xf_{half}", bufs=1) as xfpool,
                ):
                    h3 = [
                        h3pool.tile([128, L3, hb], F16, tag=f"h3g{i}", name=f"h3g{i}") for i in range(2)
                    ]
                    xf = xfpool.tile([128, hb, IL], F16)
                    if PHASES == "fc":
                        nc.vector.memset(h3[0][:], 0.0)
                        nc.vector.memset(h3[1][:], 0.0)
                        nc.vector.memset(xf[:], 0.0)

                    # ---------------- conv phase ----------------
                    with (
                        tc.tile_pool(name="xg", bufs=3) as xgpool,
                        tc.tile_pool(name="xt", bufs=3) as xtpool,
                        tc.tile_pool(name="h0", bufs=2) as h0pool,
                        tc.tile_pool(name="h1", bufs=2) as h1pool,
                        tc.tile_pool(name="h2", bufs=3) as h2pool,
                        tc.tile_pool(name="ps_xt", bufs=2, space="PSUM") as ps_xt,
                        tc.tile_pool(name="ps_h0", bufs=1, space="PSUM") as ps_h0,
                        tc.tile_pool(name="ps_h1", bufs=1, space="PSUM") as ps_h1,
                        tc.tile_pool(name="ps_h2", bufs=2, space="PSUM") as ps_h2,
                        tc.tile_pool(name="ps_h3", bufs=2, space="PSUM") as ps_h3,
                    ):
                        last_conv_mm = [None]
                        ng = hb // 4
                        if PHASES == "fc":
                            ng = 0
                        if N_GROUPS_ENV:
                            ng = min(ng, int(N_GROUPS_ENV))
                        for g in range(ng):
                            b0 = hb0 + g * 4
                            xg = xgpool.tile([CL, 4, IL], F32R)
                            nc.sync.dma_start(xg[:], xv[:, b0 : b0 + 4, :])
                            # fp16 copy of x for the FC1 x-tail
                            nc.vector.tensor_copy(xf[:, g * 4 : g * 4 + 4, :], xg[:])
                            # transposes -> xt4 [il, cl, 4]
                            xt4 = xtpool.tile([IL, CL, 4], F32R)
                            for j in range(2):
                                xt_ps = ps_xt.tile([128, 128], F32R)
                                nc.tensor.transpose(
                                    xt_ps[:], xg[:, 2 * j : 2 * j + 2, :], ident[:]
                                )
                                for jj in range(2):
                                    nc.vector.tensor_copy(
                                        xt4[:, :, 2 * j + jj],
                                        xt_ps[64 * jj : 64 * jj + 64, :],
                                    )
                            # stage0: rest + broadcast base
                            h0_ps = ps_h0.tile([PC, L0, 4], F32)
                            nc.tensor.matmul(
                                h0_ps[:], wp_sb[:, 1, :], xt4[:, 1:CL, :],
                                start=True, stop=False,
                            )
                            nc.tensor.matmul(
                                h0_ps[:], wp_sb[:, 0, :],
                                xt4[:, 0:1, :].to_broadcast([IL, L0, 4]),
                                start=False, stop=True,
                            )
                            h0 = h0pool.tile([PC, L0, 4], F32R)
                            nc.scalar.activation(
                                h0[:], h0_ps[:], AF.Relu, bias=bp_sb[:, 0:1]
                            )
                            # conv1
                            h1_ps = ps_h1.tile([CH1, L1, 4], F32)
                            for k in range(3):
                                nc.tensor.matmul(
                                    h1_ps[:], w1_sb[:, k, :], h0[:, k : k + L1, :],
                                    start=(k == 0), stop=(k == 2),
                                )
                            h1 = h1pool.tile([CH1, L1, 4], F32R)
                            nc.scalar.activation(
                                h1[:], h1_ps[:], AF.Relu, bias=b1_sb[:, 0:1]
                            )
                            # conv2 (two output chunks)
                            h2 = []
                            for oc in range(2):
                                h2_ps = ps_h2.tile([128, L2, 4], F32)
                                for k in range(3):
                                    nc.tensor.matmul(
                                        h2_ps[:],
                                        w2_sb[:, k, oc * 128 : (oc + 1) * 128],
                                        h1[:, k : k + L2, :],
                                        start=(k == 0), stop=(k == 2),
                                    )
                                t = h2pool.tile([128, L2, 4], F32R)
                                nc.vector.tensor_scalar(
                                    t[:], h2_ps[:],
                                    scalar1=b2_sb[:, oc : oc + 1], scalar2=0.0,
                                    op0=ALU.add, op1=ALU.max,
                                )
                                h2.append(t)
                            # conv3 (contract 256 ch) -> resident fp16 h3
                            for oc in range(2):
                                h3_ps = ps_h3.tile([128, L3, 4], F32)
                                n = 0
                                for cg in range(2):
                                    for k in range(3):
                                        last_conv_mm[0] = nc.tensor.matmul(
                                            h3_ps[:],
                                            w3_sb[:, k, cg, oc * 128 : (oc + 1) * 128],
                                            h2[cg][:, k : k + L3, :],
                                            start=(n == 0), stop=(n == 5),
                                        )
                                        n += 1
                                dst = h3[oc][:, :, g * 4 : g * 4 + 4]
                                if oc == 0:
                                    nc.vector.tensor_scalar(
                                        dst, h3_ps[:],
                                        scalar1=b3_sb[:, 0:1], scalar2=0.0,
                                        op0=ALU.add, op1=ALU.max,
                                    )
                                else:
                                    nc.scalar.activation(
                                        dst, h3_ps[:], AF.Relu, bias=b3_sb[:, 1:2]
                                    )

                    # ---------------- FC phase ----------------
                    with (
                        tc.tile_pool(name="wst", bufs=6) as wpool,
                        tc.tile_pool(name="h4p", bufs=1) as h4pool,
                        tc.tile_pool(name="osb", bufs=1) as opool,
                        tc.tile_pool(name="ps_fc", bufs=1, space="PSUM") as ps_fc,
                        tc.tile_pool(name="ps_o", bufs=1, space="PSUM") as ps_o,
                    ):
                        # Non-FWL fp16 interposer matmul between the f32r conv
                        # matmuls and the FWL-enabled fp16 FC matmuls (the FWL
                        # weight path hangs the PE when it directly follows a
                        # 4-byte matmul; NumWeights==1 keeps FWL off here).
                        flush_ps = ps_o.tile([1, 1], F32, name="flushps", tag="flush")
                        flush_mm = nc.tensor.matmul(
                            flush_ps[:], h3[0][:, 0:1, 0], h3[0][:, 0:1, 0],
                            start=True, stop=True,
                        )
                        if last_conv_mm[0] is not None:
                            add_dep_helper(
                                flush_mm.ins, last_conv_mm[0].ins, sync=False,
                                reason="interposer after last conv matmul",
                            )
                        first_fc_mm = [None]
                        fc_ps = [
                            ps_fc.tile([NF_CH[c], hb], F32, tag=f"fc{c}", name=f"fc{c}")
                            for c in range(4)
                        ]
                        n_il = 0 if PHASES == "conv" else IL
                        n_pos = 0 if PHASES == "conv" else L3
                        n_acc = n_il + n_pos * 2  # accumulation steps per chunk
                        step = 0
                        if PHASES == "conv":
                            for c in range(4):
                                nc.vector.memset(fc_ps[c][:], 0.0)
                        for il in range(n_il):
                            wt = wpool.tile([CL, F1], F16, tag="w")
                            nc.sync.dma_start(wt[:], wf1x_d[il])
                            for c in range(4):
                                cs, ofs = NF_CH[c], NF_OFS[c]
                                mm = nc.tensor.matmul(
                                    fc_ps[c][:],
                                    wt[:, ofs : ofs + cs],
                                    xf[:, :, il],
                                    start=(step == 0), stop=(step == n_acc - 1),
                                )
                                if first_fc_mm[0] is None:
                                    first_fc_mm[0] = mm
                                    add_dep_helper(
                                        mm.ins, flush_mm.ins, sync=False,
                                        reason="order FWL mm after interposer",
                                    )
                            step += 1
                        for pos in range(n_pos):
                            for grp in range(2):
                                wt = wpool.tile([128, F1], F16, tag="w")
                                nc.sync.dma_start(wt[:], wf1c_d[pos, grp])
                                for c in range(4):
                                    cs, ofs = NF_CH[c], NF_OFS[c]
                                    nc.tensor.matmul(
                                        fc_ps[c][:],
                                        wt[:, ofs : ofs + cs],
                                        h3[grp][:, pos, :],
                                        start=(step == 0), stop=(step == n_acc - 1),
                                    )
                                step += 1

                        # FC1 bias+relu, then FC2
                        out_ps = ps_o.tile([1, hb], F32)
                        for c in range(4):
                            cs = NF_CH[c]
                            h4 = h4pool.tile([cs, hb], F16, tag=f"h4{c}")
                            nc.vector.tensor_scalar(
                                h4[:], fc_ps[c][:],
                                scalar1=bf1_sb[0:cs, c : c + 1], scalar2=0.0,
                                op0=ALU.add, op1=ALU.max,
                            )
                            nc.tensor.matmul(
                                out_ps[:],
                                wf2_sb[0:cs, c : c + 1],
                                h4[:],
                                start=(c == 0), stop=(c == 3),
                            )
                        out_sb = opool.tile([1, hb], F32)
                        nc.vector.tensor_scalar(
                            out_sb[:], out_ps[:],
                            scalar1=bf2_sb[0:1, 0:1], scalar2=None, op0=ALU.add,
                        )
                        nc.sync.dma_start(
                            out_d[hb0 : hb0 + hb, :].rearrange("b o -> o b"),
                            out_sb[:],
                        )
    nc.finalize()
    return nc


def prep_weights(Wp, bp, W1, b1, W2, b2, W3, b3, Wf1, bf1, Wf2, bf2):
    wf1c = Wf1[:, :C3_OUT].reshape(F1, 2, 128, L3).transpose(3, 1, 2, 0)
    wf1x = Wf1[:, C3_OUT:].reshape(F1, IL, CL).transpose(1, 2, 0)
    return dict(
        ident=np.eye(128, dtype=np.float32),
        wp=_r(np.transpose(Wp, (2, 1, 0))),
        bp=_r(bp.reshape(PC, 1)),
        w1=_r(np.transpose(W1, (2, 1, 0))),
        b1=_r(b1.reshape(CH1, 1)),
        w2=_r(np.transpose(W2, (2, 1, 0))),
        b2=_r(b2.reshape(CH2, 1)),
        w3=_r(np.transpose(W3, (2, 1, 0)).reshape(3, 2, 128, CH3)),
        b3=_r(b3.reshape(CH3, 1)),
        wf1c=_h(wf1c),
        wf1x=_h(wf1x),
        bf1=_r(bf1.reshape(F1, 1)),
        wf2=_h(Wf2.reshape(F1, 1)),
        bf2=_r(np.asarray(bf2, np.float32).reshape(1, 1)),
    )


_RUN_KW = {}  # test harness may add trace=True


def kernel(x, Wp, bp, W1, b1, W2, b2, W3, b3, Wf1, bf1, Wf2, bf2):
    x = np.ascontiguousarray(np.asarray(x, np.float32))
    weights = prep_weights(
        np.asarray(Wp), np.asarray(bp), np.asarray(W1), np.asarray(b1),
        np.asarray(W2), np.asarray(b2), np.asarray(W3), np.asarray(b3),
        np.asarray(Wf1), np.asarray(bf1), np.asarray(Wf2), np.asarray(bf2),
    )
    nc = build_program()
    in_maps = [
        {**weights, "x": x[i * B_SH : (i + 1) * B_SH]} for i in range(N_CORES)
    ]
    res = run_bass_kernel_spmd(nc, in_maps, list(range(N_CORES)), **_RUN_KW)
    out = np.concatenate([res.results[i]["out"] for i in range(N_CORES)], axis=0)
    if _RUN_KW:
        kernel.last_results = res
    return out


if __name__ == "__main__":
    import reference

    inputs = {k: np.asarray(v) for k, v in reference.setup_inputs().items()}
    want = np.asarray(reference.reference(**inputs))
    got = kernel(**inputs)
    err = np.abs(got - want).max() / (np.abs(want).max() + 1e-12)
    print("rel err:", err)


# revision 14
# speedup vs baseline: 26.1250x; 26.1250x over previous
"""Trainium2 Bass kernel for nn_CNN3_P_lat (dense CNN, 8-core data parallel).

Network (per sample): pairwise-conv stage0 -> 3x conv1d(k=3) -> flatten+concat
with x-transpose -> FC(39168->400) -> relu -> FC(400->1).

Strategy:
  - Pure data parallel: 512 samples per core, weights replicated.
  - Conv stack in float32r (fp32 data, FP22 multiply) at full PE rate; layout
    is channels-on-partitions, (position, batch-of-4) on the free dim.
  - Per-sample transpose of x built on the fly with PE transposes.
  - conv3 output (h3) kept RESIDENT in SBUF as fp16, batch processed in two
    halves of 256 samples so it fits; FC1 runs as 121 position-sliced fp16
    matmul-accumulates (weights pre-permuted on host) plus 64 cl-sliced
    matmuls for the concatenated x tail (contract over cl).
  - FC2 folded onto the same psum pass.
"""

import os
import sys

sys.path.insert(0, "/opt/trn_rl_repo")

PHASES = os.environ.get("KPHASES", "all")  # all | conv | fc
N_GROUPS_ENV = os.environ.get("KGROUPS")  # limit conv groups for bisection
KREPEAT = int(os.environ.get("KREPEAT", "1"))  # repeat whole computation (timing)

import numpy as np

import concourse.bass as bass
from concourse import bacc
import concourse.mybir as mybir
import concourse.tile as tile
from concourse.bass_utils import run_bass_kernel_spmd
from concourse.tile_rust import add_dep_helper

F32 = mybir.dt.float32
F32R = mybir.dt.float32r
F16 = mybir.dt.float16
AF = mybir.ActivationFunctionType
ALU = mybir.AluOpType

N_CORES = 8
B = 4096
B_SH = B // N_CORES  # 512 samples per core
HB = 256  # half-batch (h3 residency granularity)
CL, IL = 128, 64
PC, CH1, CH2, CH3 = 64, 128, 256, 256
L0, L1, L2, L3 = 127, 125, 123, 121
F1 = 400
C3_OUT = CH3 * L3  # 30976

NF_CH = [128, 128, 128, 16]  # FC1 output chunking (400 total)
NF_OFS = [0, 128, 256, 384]


def _r(a):
    return np.ascontiguousarray(a, dtype=np.float32)


def _h(a):
    return np.ascontiguousarray(a, dtype=np.float16)


def build_program(b_sh=B_SH, hb=HB, repeat=None):
    repeat = KREPEAT if repeat is None else repeat
    nc = bacc.Bacc("TRN2", target_bir_lowering=False)

    x_d = nc.dram_tensor("x", [b_sh, CL * IL], F32, kind="ExternalInput")
    id_d = nc.dram_tensor("ident", [128, 128], F32, kind="ExternalInput")
    wp_d = nc.dram_tensor("wp", [2, IL, PC], F32, kind="ExternalInput")
    bp_d = nc.dram_tensor("bp", [PC, 1], F32, kind="ExternalInput")
    w1_d = nc.dram_tensor("w1", [3, PC, CH1], F32, kind="ExternalInput")
    b1_d = nc.dram_tensor("b1", [CH1, 1], F32, kind="ExternalInput")
    w2_d = nc.dram_tensor("w2", [3, CH1, CH2], F32, kind="ExternalInput")
    b2_d = nc.dram_tensor("b2", [CH2, 1], F32, kind="ExternalInput")
    w3_d = nc.dram_tensor("w3", [3, 2, 128, CH3], F32, kind="ExternalInput")
    b3_d = nc.dram_tensor("b3", [CH3, 1], F32, kind="ExternalInput")
    wf1c_d = nc.dram_tensor("wf1c", [L3, 2, 128, F1], F16, kind="ExternalInput")
    wf1x_d = nc.dram_tensor("wf1x", [IL, CL, F1], F16, kind="ExternalInput")
    bf1_d = nc.dram_tensor("bf1", [F1, 1], F32, kind="ExternalInput")
    wf2_d = nc.dram_tensor("wf2", [F1, 1], F16, kind="ExternalInput")
    bf2_d = nc.dram_tensor("bf2", [1, 1], F32, kind="ExternalInput")
    out_d = nc.dram_tensor("out", [b_sh, 1], F32, kind="ExternalOutput")

    with tile.TileContext(nc) as tc:
        with tc.tile_pool(name="const", bufs=1) as cpool:
            ident = cpool.tile([128, 128], F32R)
            nc.sync.dma_start(ident[:], id_d[:].bitcast(F32R))
            wp_sb = cpool.tile([IL, 2, PC], F32R)
            nc.sync.dma_start(wp_sb[:], wp_d.rearrange("k i p -> i k p").bitcast(F32R))
            w1_sb = cpool.tile([PC, 3, CH1], F32R)
            nc.sync.dma_start(w1_sb[:], w1_d.rearrange("k c o -> c k o").bitcast(F32R))
            w2_sb = cpool.tile([CH1, 3, CH2], F32R)
            nc.sync.dma_start(w2_sb[:], w2_d.rearrange("k c o -> c k o").bitcast(F32R))
            w3_sb = cpool.tile([128, 3, 2, CH3], F32R)
            nc.sync.dma_start(
                w3_sb[:], w3_d.rearrange("k g c o -> c k g o").bitcast(F32R)
            )
            bp_sb = cpool.tile([PC, 1], F32)
            nc.sync.dma_start(bp_sb[:], bp_d[:])
            b1_sb = cpool.tile([CH1, 1], F32)
            nc.sync.dma_start(b1_sb[:], b1_d[:])
            b2_sb = cpool.tile([128, 2], F32)
            nc.sync.dma_start(b2_sb[:], b2_d.rearrange("(g c) o -> c g o", c=128))
            b3_sb = cpool.tile([128, 2], F32)
            nc.sync.dma_start(b3_sb[:], b3_d.rearrange("(g c) o -> c g o", c=128))
            bf1_sb = cpool.tile([128, 4], F32)
            wf2_sb = cpool.tile([128, 4], F16)
            for c in range(4):
                cs, ofs = NF_CH[c], NF_OFS[c]
                nc.sync.dma_start(bf1_sb[0:cs, c : c + 1], bf1_d[ofs : ofs + cs, :])
                nc.sync.dma_start(wf2_sb[0:cs, c : c + 1], wf2_d[ofs : ofs + cs, :])
            bf2_sb = cpool.tile([1, 1], F32)
            nc.sync.dma_start(bf2_sb[:], bf2_d[:])

            xv = x_d.rearrange("b (cl il) -> cl b il", il=IL).bitcast(F32R)

            def emit_whole():
              for half in range(b_sh // hb):
                hb0 = half * hb
                with (
                    tc.tile_pool(name=f"h3_{half}", bufs=1) as h3pool,
                    tc.tile_pool(name=f"xf_{half}", bufs=1) as xfpool,
                ):
                    h3 = [
                        h3pool.tile([128, L3, hb], F16, tag=f"h3g{i}", name=f"h3g{i}") for i in range(2)
                    ]
                    xf = xfpool.tile([128, hb, IL], F16)
                    if PHASES == "fc":
                        nc.vector.memset(h3[0][:], 0.0)
                        nc.vector.memset(h3[1][:], 0.0)
                        nc.vector.memset(xf[:], 0.0)

                    # ---------------- conv phase ----------------
                    with (
                        tc.tile_pool(name="xg", bufs=3) as xgpool,
                        tc.tile_pool(name="xt", bufs=3) as xtpool,
                        tc.tile_pool(name="h0", bufs=2) as h0pool,
                        tc.tile_pool(name="h1", bufs=2) as h1pool,
                        tc.tile_pool(name="h2", bufs=3) as h2pool,
                        tc.tile_pool(name="ps_xt", bufs=2, space="PSUM") as ps_xt,
                        tc.tile_pool(name="ps_h0", bufs=1, space="PSUM") as ps_h0,
                        tc.tile_pool(name="ps_h1", bufs=1, space="PSUM") as ps_h1,
                        tc.tile_pool(name="ps_h2", bufs=2, space="PSUM") as ps_h2,
                        tc.tile_pool(name="ps_h3", bufs=2, space="PSUM") as ps_h3,
                    ):
                        last_conv_mm = [None]
                        ng = hb // 4
                        if PHASES == "fc":
                            ng = 0
                        if N_GROUPS_ENV:
                            ng = min(ng, int(N_GROUPS_ENV))
                        for g in range(ng):
                            b0 = hb0 + g * 4
                            xg = xgpool.tile([CL, 4, IL], F32R)
                            nc.sync.dma_start(xg[:], xv[:, b0 : b0 + 4, :])
                            # fp16 copy of x for the FC1 x-tail
                            nc.vector.tensor_copy(xf[:, g * 4 : g * 4 + 4, :], xg[:])
                            # transposes -> xt4 [il, cl, 4]
                            xt4 = xtpool.tile([IL, CL, 4], F32R)
                            for j in range(2):
                                xt_ps = ps_xt.tile([128, 128], F32R)
                                nc.tensor.transpose(
                                    xt_ps[:], xg[:, 2 * j : 2 * j + 2, :], ident[:]
                                )
                                for jj in range(2):
                                    nc.vector.tensor_copy(
                                        xt4[:, :, 2 * j + jj],
                                        xt_ps[64 * jj : 64 * jj + 64, :],
                                    )
                            # stage0: rest + broadcast base
                            h0_ps = ps_h0.tile([PC, L0, 4], F32)
                            nc.tensor.matmul(
                                h0_ps[:], wp_sb[:, 1, :], xt4[:, 1:CL, :],
                                start=True, stop=False,
                            )
                            nc.tensor.matmul(
                                h0_ps[:], wp_sb[:, 0, :],
                                xt4[:, 0:1, :].to_broadcast([IL, L0, 4]),
                                start=False, stop=True,
                            )
                            h0 = h0pool.tile([PC, L0, 4], F32R)
                            nc.scalar.activation(
                                h0[:], h0_ps[:], AF.Relu, bias=bp_sb[:, 0:1]
                            )
                            # conv1
                            h1_ps = ps_h1.tile([CH1, L1, 4], F32)
                            for k in range(3):
                                nc.tensor.matmul(
                                    h1_ps[:], w1_sb[:, k, :], h0[:, k : k + L1, :],
                                    start=(k == 0), stop=(k == 2),
                                )
                            h1 = h1pool.tile([CH1, L1, 4], F32R)
                            nc.scalar.activation(
                                h1[:], h1_ps[:], AF.Relu, bias=b1_sb[:, 0:1]
                            )
                            # conv2 (two output chunks)
                            h2 = []
                            for oc in range(2):
                                h2_ps = ps_h2.tile([128, L2, 4], F32)
                                for k in range(3):
                                    nc.tensor.matmul(
                                        h2_ps[:],
                                        w2_sb[:, k, oc * 128 : (oc + 1) * 128],
                                        h1[:, k : k + L2, :],
                                        start=(k == 0), stop=(k == 2),
                                    )
                                t = h2pool.tile([128, L2, 4], F32R)
                                nc.vector.tensor_scalar(
                                    t[:], h2_ps[:],
                                    scalar1=b2_sb[:, oc : oc + 1], scalar2=0.0,
                                    op0=ALU.add, op1=ALU.max,
                                )
                                h2.append(t)
                            # conv3 (contract 256 ch) -> resident fp16 h3
                            for oc in range(2):
                                h3_ps = ps_h3.tile([128, L3, 4], F32)
                                n = 0
                                for cg in range(2):
                                    for k in range(3):
                                        last_conv_mm[0] = nc.tensor.matmul(
                                            h3_ps[:],
                                            w3_sb[:, k, cg, oc * 128 : (oc + 1) * 128],
                                            h2[cg][:, k : k + L3, :],
                                            start=(n == 0), stop=(n == 5),
                                        )
                                        n += 1
                                dst = h3[oc][:, :, g * 4 : g * 4 + 4]
                                if oc == 0:
                                    nc.vector.tensor_scalar(
                                        dst, h3_ps[:],
                                        scalar1=b3_sb[:, 0:1], scalar2=0.0,
                                        op0=ALU.add, op1=ALU.max,
                                    )
                                else:
                                    nc.scalar.activation(
                                        dst, h3_ps[:], AF.Relu, bias=b3_sb[:, 1:2]
                                    )

                    # ---------------- FC phase ----------------
                    with (
                        tc.tile_pool(name="wst", bufs=6) as wpool,
                        tc.tile_pool(name="h4p", bufs=1) as h4pool,
                        tc.tile_pool(name="osb", bufs=1) as opool,
                        tc.tile_pool(name="ps_fc", bufs=1, space="PSUM") as ps_fc,
                        tc.tile_pool(name="ps_o", bufs=1, space="PSUM") as ps_o,
                    ):
                        # Non-FWL fp16 interposer matmul between the f32r conv
                        # matmuls and the FWL-enabled fp16 FC matmuls (the FWL
                        # weight path hangs the PE when it directly follows a
                        # 4-byte matmul; NumWeights==1 keeps FWL off here).
                        flush_ps = ps_o.tile([1, 1], F32, name="flushps", tag="flush")
                        flush_mm = nc.tensor.matmul(
                            flush_ps[:], h3[0][:, 0:1, 0], h3[0][:, 0:1, 0],
                            start=True, stop=True,
                        )
                        if last_conv_mm[0] is not None:
                            add_dep_helper(
                                flush_mm.ins, last_conv_mm[0].ins, sync=False,
                                reason="interposer after last conv matmul",
                            )
                        first_fc_mm = [None]
                        fc_ps = [
                            ps_fc.tile([NF_CH[c], hb], F32, tag=f"fc{c}", name=f"fc{c}")
                            for c in range(4)
                        ]
                        n_il = 0 if PHASES == "conv" else IL
                        n_pos = 0 if PHASES == "conv" else L3
                        n_acc = n_il + n_pos * 2  # accumulation steps per chunk
                        step = 0
                        if PHASES == "conv":
                            for c in range(4):
                                nc.vector.memset(fc_ps[c][:], 0.0)
                        for il in range(n_il):
                            wt = wpool.tile([CL, F1], F16, tag="w")
                            nc.sync.dma_start(wt[:], wf1x_d[il])
                            for c in range(4):
                                cs, ofs = NF_CH[c], NF_OFS[c]
                                mm = nc.tensor.matmul(
                                    fc_ps[c][:],
                                    wt[:, ofs : ofs + cs],
                                    xf[:, :, il],
                                    start=(step == 0), stop=(step == n_acc - 1),
                                )
                                if first_fc_mm[0] is None:
                                    first_fc_mm[0] = mm
                                    add_dep_helper(
                                        mm.ins, flush_mm.ins, sync=False,
                                        reason="order FWL mm after interposer",
                                    )
                            step += 1
                        for pos in range(n_pos):
                            for grp in range(2):
                                wt = wpool.tile([128, F1], F16, tag="w")
                                nc.sync.dma_start(wt[:], wf1c_d[pos, grp])
                                for c in range(4):
                                    cs, ofs = NF_CH[c], NF_OFS[c]
                                    nc.tensor.matmul(
                                        fc_ps[c][:],
                                        wt[:, ofs : ofs + cs],
                                        h3[grp][:, pos, :],
                                        start=(step == 0), stop=(step == n_acc - 1),
                                    )
                                step += 1

                        # FC1 bias+relu, then FC2
                        out_ps = ps_o.tile([1, hb], F32)
                        for c in range(4):
                            cs = NF_CH[c]
                            h4 = h4pool.tile([cs, hb], F16, tag=f"h4{c}")
                            nc.vector.tensor_scalar(
                                h4[:], fc_ps[c][:],
                                scalar1=bf1_sb[0:cs, c : c + 1], scalar2=0.0,
                                op0=ALU.add, op1=ALU.max,
                            )
                            nc.tensor.matmul(
                                out_ps[:],
                                wf2_sb[0:cs, c : c + 1],
                                h4[:],
                                start=(c == 0), stop=(c == 3),
                            )
                        out_sb = opool.tile([1, hb], F32)
                        nc.vector.tensor_scalar(
                            out_sb[:], out_ps[:],
                            scalar1=bf2_sb[0:1, 0:1], scalar2=None, op0=ALU.add,
                        )
                        nc.sync.dma_start(
                            out_d[hb0 : hb0 + hb, :].rearrange("b o -> o b"),
                            out_sb[:],
                        )
            if repeat > 1:
                with tc.For_i(0, repeat, 1):
                    emit_whole()
            else:
                emit_whole()
    nc.finalize()
    return nc


def prep_weights(Wp, bp, W1, b1, W2, b2, W3, b3, Wf1, bf1, Wf2, bf2):
    wf1c = Wf1[:, :C3_OUT].reshape(F1, 2, 128, L3).transpose(3, 1, 2, 0)
    wf1x = Wf1[:, C3_OUT:].reshape(F1, IL, CL).transpose(1, 2, 0)
    return dict(
        ident=np.eye(128, dtype=np.float32),
        wp=_r(np.transpose(Wp, (2, 1, 0))),
        bp=_r(bp.reshape(PC, 1)),
        w1=_r(np.transpose(W1, (2, 1, 0))),
        b1=_r(b1.reshape(CH1, 1)),
        w2=_r(np.transpose(W2, (2, 1, 0))),
        b2=_r(b2.reshape(CH2, 1)),
        w3=_r(np.transpose(W3, (2, 1, 0)).reshape(3, 2, 128, CH3)),
        b3=_r(b3.reshape(CH3, 1)),
        wf1c=_h(wf1c),
        wf1x=_h(wf1x),
        bf1=_r(bf1.reshape(F1, 1)),
        wf2=_h(Wf2.reshape(F1, 1)),
        bf2=_r(np.asarray(bf2, np.float32).reshape(1, 1)),
    )


_RUN_KW = {}  # test harness may add trace=True


def kernel(x, Wp, bp, W1, b1, W2, b2, W3, b3, Wf1, bf1, Wf2, bf2):
    x = np.ascontiguousarray(np.asarray(x, np.float32))
    weights = prep_weights(
        np.asarray(Wp), np.asarray(bp), np.asarray(W1), np.asarray(b1),
        np.asarray(W2), np.asarray(b2), np.asarray(W3), np.asarray(b3),
        np.asarray(Wf1), np.asarray(bf1), np.asarray(Wf2), np.asarray(bf2),
    )
    nc = build_program()
    in_maps = [
        {**weights, "x": x[i * B_SH : (i + 1) * B_SH]} for i in range(N_CORES)
    ]
    res = run_bass_kernel_spmd(nc, in_maps, list(range(N_CORES)), **_RUN_KW)
    out = np.concatenate([res.results[i]["out"] for i in range(N_CORES)], axis=0)
    if _RUN_KW:
        kernel.last_results = res
    return out


if __name__ == "__main__":
    import reference

    inputs = {k: np.asarray(v) for k, v in reference.setup_inputs().items()}
    want = np.asarray(reference.reference(**inputs))
    got = kernel(**inputs)
    err = np.abs(got - want).max() / (np.abs(want).max() + 1e-12)
    print("rel err:", err)
